# revision 1
# baseline (speedup 1.0000x reference)
"""Trainium2 Bass kernel for nn_BinaryGroupConv.

Reference op (per image): BatchNorm2d (inference) -> sign-binarize ->
grouped 3x3 conv (64 groups, 4->4 ch, binarized weights) -> channel
shuffle -> residual add.

Strategy:
  - Data-parallel: 32 images / 8 cores = 4 images per core. No collectives.
  - BN+sign on device: DVE tensor_scalar (x*inv then +t, separately rounded,
    bit-identical to the eager jax reference) then ACT Sign -> bf16 (+-1 and 0
    are exact in bf16).
  - Grouped conv as 9 per-tap block-diagonal matmuls [K=128ch, M=128ch,
    N=464 spatial] accumulated in PSUM. Signed values are exact in bf16, so
    the PE runs at full bf16 rate. Activations are stored in a zero-padded
    58x58 layout so all 9 taps are plain column-shifted slices of one SBUF
    buffer (row-pad columns absorb the horizontal wrap).
  - Channel shuffle is folded into the matmul output-column permutation
    (psum partition m = 32*(c%4) + ((c//4)-32*chunk)), which makes both the
    residual-x gather and the output store simple 32-channel-contiguous DMAs.
  - Residual add fused with the PSUM->SBUF drain on DVE.
"""

import numpy as np

import jax
import ml_dtypes

import concourse.bacc as bacc
import concourse.tile as tile
from concourse import mybir
from contextlib import ExitStack

N_CORES = 8
IMG = 4           # images per core
C = 256
H = W = 56
HP = 58           # padded row length
GRID = HP * HP    # 58x58 padded image
APAD = GRID + 2   # +1 guard element on each side
ROWS_PER_TILE = 8
NT = H // ROWS_PER_TILE          # 7 output tiles per image-chunk
TN = ROWS_PER_TILE * HP          # 464 matmul free dim
TN56 = ROWS_PER_TILE * W         # 448 valid columns per tile
EPS = 1e-5
RHS_MODE = "strided"  # "strided" (N=448, pad cols skipped) or "padded" (N=464)

_CACHE = {}


def _build_program(repeat=1):
    nc = bacc.Bacc("TRN2")
    f32 = mybir.dt.float32
    bf16 = mybir.dt.bfloat16
    x_in = nc.declare_dram_parameter("x", [IMG, C, H, W], f32, isOutput=False)
    wt_in = nc.declare_dram_parameter("wt", [128, 18 * 128], bf16, isOutput=False)
    bn_in = nc.declare_dram_parameter("bn", [128, 4], f32, isOutput=False)
    y_out = nc.declare_dram_parameter("y", [IMG, C, H, W], f32, isOutput=True)

    with tile.TileContext(nc) as tc, ExitStack() as ctx:
        const_pool = ctx.enter_context(tc.tile_pool(name="const", bufs=1))
        apad_pool = ctx.enter_context(tc.tile_pool(name="apad", bufs=1))
        x_pool = ctx.enter_context(tc.tile_pool(name="xin", bufs=3))
        xp_pool = ctx.enter_context(tc.tile_pool(name="xp", bufs=3))
        psum_pool = ctx.enter_context(
            tc.tile_pool(name="ps", bufs=4, space="PSUM")
        )

        # Trigger the ACT table load (Sign/Identity set, ~2.7us) immediately
        # so it overlaps the first DMAs instead of the first real activation.
        warm = const_pool.tile([128, 2], f32, tag="actwarm")
        nc.vector.memset(warm[:], 0.0)
        nc.scalar.activation(warm[:], warm[:], mybir.ActivationFunctionType.Sign)

        bn_sb = const_pool.tile([128, 4], f32, tag="bn")
        nc.sync.dma_start(bn_sb[:], bn_in[:])
        wt_sb = const_pool.tile([128, 18 * 128], bf16, tag="wt")

        apads = []
        for b in range(3):
            ap_t = apad_pool.tile([128, APAD], bf16, tag=f"apad{b}")
            # Zero only the pad cells; ACT rewrites the interior every use.
            nc.vector.memset(ap_t[:, 0:59], 0.0)  # guard + top pad row
            nc.vector.memset(  # right pad of row r | left pad of row r+1 pairs
                ap_t[:, 58 : 58 + 57 * HP].rearrange("p (r z) -> p r z", z=HP)[
                    :, :, 0:2
                ],
                0.0,
            )
            nc.vector.memset(ap_t[:, 1 + 57 * HP : APAD], 0.0)  # bottom + guard
            apads.append(ap_t)

        # Software pipeline, prefetch depth 2: loads(k+1) are emitted a full
        # chunk ahead of bnsign(k) and compute(k-1), so the next chunk's x
        # never queues behind bulk xp/store traffic in the DMA FIFO.
        chunks = [
            (img, c)
            for _rep in range(repeat)
            for img in range(IMG)
            for c in range(2)
        ]
        nc.sync.dma_start(wt_sb[:], wt_in[:])
        signed = [None] * len(chunks)
        for k in range(len(chunks)):
            signed[k] = _emit_prep(nc, k, chunks[k], x_in, bn_sb, apads,
                                   x_pool, xp_pool)
            if k >= 1:
                _emit_compute(nc, signed[k - 1], y_out, wt_sb, psum_pool)
        _emit_compute(nc, signed[-1], y_out, wt_sb, psum_pool, last=True)
    nc.compile()
    return nc


def _emit_prep(nc, k, chunk, x_in, bn_sb, apads, x_pool, xp_pool):
    img, c = chunk
    f32 = mybir.dt.float32
    ap_t = apads[k % 3]
    x_t = x_pool.tile([128, H * W], f32, tag="x")
    # Row-piece pipeline: load / BN-affine / sign per piece, so the first
    # rows' matmuls can start while later rows still prep. Both ACT ops are
    # single-rounded fmas replicating the eager reference's separate mul
    # then add: bit-exact end to end. Chunk 0 preps in quarters so the very
    # first matmuls start ASAP.
    pieces = 7  # 8-row pieces, exactly one output tile each
    rows = H // pieces
    for hh in range(pieces):
        r0 = hh * rows
        sl = slice(r0 * W, (r0 + rows) * W)
        nc.sync.dma_start(
            x_t[:, sl],
            x_in[img, 128 * c : 128 * (c + 1), r0 : r0 + rows, :].rearrange(
                "c h w -> c (h w)"
            ),
        )
        # y = RN(x*inv): fma with zero addend. Keeps DVE free for the adds.
        nc.scalar.activation(
            x_t[:, sl],
            x_t[:, sl],
            mybir.ActivationFunctionType.Identity,
            bias=0.0,
            scale=bn_sb[:, 2 * c : 2 * c + 1],
        )
        interior = ap_t[
            :, 1 + HP * (r0 + 1) + 1 : 1 + HP * (r0 + 1) + 1 + rows * HP
        ].rearrange("p (h w) -> p h w", w=HP)[:, :, 0:W]
        nc.scalar.activation(
            interior,
            x_t[:, sl].rearrange("p (h w) -> p h w", w=W),
            mybir.ActivationFunctionType.Sign,
            bias=bn_sb[:, 2 * c + 1 : 2 * c + 2],
            scale=1.0,
        )
    # Residual x in shuffled channel order (partition 32i+q <- channel
    # 64i+32c+q): 4 large contiguous DMAs. The DVE add accumulates the conv
    # result into this buffer in place; it then doubles as the store source.
    xp = xp_pool.tile([128, H * W], f32, tag="xp")
    for i in range(4):
        ch0 = 64 * i + 32 * c
        nc.sync.dma_start(
            xp[32 * i : 32 * i + 32, :],
            x_in[img, ch0 : ch0 + 32, :, :].rearrange("c h w -> c (h w)"),
        )
    return (img, c, ap_t, xp)


def _emit_compute(nc, stage, y_out, wt_sb, psum_pool, last=False):
    img, c, ap_t, xp = stage
    f32 = mybir.dt.float32
    # Store column groups as soon as their adds are done.
    store_after = {3: (0, 32), 6: (32, 56)}
    ap_grid = ap_t[:, 1 : 1 + HP * HP].rearrange("p (h w) -> p h w", w=HP)
    for t in range(NT):
        if RHS_MODE == "strided":
            ps = psum_pool.tile([128, TN56], f32, tag="ps")
            for tap in range(9):
                dh, dw = tap // 3 - 1, tap % 3 - 1
                r0 = ROWS_PER_TILE * t + 1 + dh
                rows = ROWS_PER_TILE
                o0 = 0
                # Edge tiles: the tap row falling entirely on the zero pad
                # row contributes nothing -- skip it. Partial-width matmuls
                # are safe: start=True clears the whole bank's has_written
                # bits, and later taps overwrite columns not yet written.
                if t == 0 and dh == -1:
                    r0, rows, o0 = r0 + 1, rows - 1, W
                elif t == NT - 1 and dh == 1:
                    rows = rows - 1
                nc.tensor.matmul(
                    ps[:, o0 : o0 + rows * W],
                    wt_sb[:, (9 * c + tap) * 128 : (9 * c + tap + 1) * 128],
                    ap_grid[:, r0 : r0 + rows, 1 + dw : 1 + dw + W],
                    start=(tap == 0),
                    stop=(tap == 8),
                )
            ps_v = ps[:]
        else:
            ps = psum_pool.tile([128, TN], f32, tag="ps")
            for tap in range(9):
                dh, dw = tap // 3 - 1, tap % 3 - 1
                s = 1 + HP * (ROWS_PER_TILE * t + 1 + dh) + dw
                nc.tensor.matmul(
                    ps[:],
                    wt_sb[:, (9 * c + tap) * 128 : (9 * c + tap + 1) * 128],
                    ap_t[:, s : s + TN],
                    start=(tap == 0),
                    stop=(tap == 8),
                )
            ps_v = ps.rearrange("p (h w) -> p h w", w=HP)[:, :, 1 : 1 + W]
        xp_v = xp[:, TN56 * t : TN56 * (t + 1)]
        if RHS_MODE != "strided":
            xp_v = xp_v.rearrange("p (h w) -> p h w", w=W)
        nc.vector.tensor_tensor(xp_v, ps_v, xp_v, op=mybir.AluOpType.add)
        if t in store_after:
            r0, r1 = store_after[t]
            for i in range(4):
                ch0 = 64 * i + 32 * c
                nc.sync.dma_start(
                    y_out[img, ch0 : ch0 + 32, r0:r1, :].rearrange(
                        "c h w -> c (h w)"
                    ),
                    xp[32 * i : 32 * i + 32, r0 * W : r1 * W],
                )


def _pack_weights(weight):
    """Block-diagonal per-tap lhsT tiles with shuffle-folded output order.

    wt[k, (9c+tap)*128 + m]: psum partition m = 32*i + q holds conv output
    channel oc = 128c + 4q + i (group q of chunk c). Nonzero iff input row
    k is in group q (k//4 == q), value sign(weight[oc, k%4, kh, kw]).
    """
    ws = np.sign(weight.astype(np.float32))  # [256, 4, 3, 3]
    wt = np.zeros((128, 2, 9, 128), np.float32)
    q = np.arange(32)
    for c in range(2):
        for tap in range(9):
            kh, kw = tap // 3, tap % 3
            # arr[q, i, j] = ws[128c + 4q + i, j, kh, kw]
            arr = ws[128 * c : 128 * (c + 1), :, kh, kw].reshape(32, 4, 4)
            B = np.zeros((32, 4, 4, 32), np.float32)  # [q, j, i, q']
            B[q, :, :, q] = arr.transpose(0, 2, 1)
            wt[:, c, tap, :] = B.reshape(128, 128)
    return wt.reshape(128, 18 * 128).astype(ml_dtypes.bfloat16)


def _pack_bn(gamma, beta, running_mean, running_var):
    # Mirror the reference ops (and platform) bit-for-bit.
    import jax.numpy as jnp

    inv = np.asarray(
        jnp.asarray(gamma) * jax.lax.rsqrt(jnp.asarray(running_var) + EPS)
    )
    t = np.asarray(jnp.asarray(beta) - jnp.asarray(running_mean) * jnp.asarray(inv))
    bn = np.zeros((128, 4), np.float32)
    bn[:, 0] = inv[0:128]
    bn[:, 1] = t[0:128]
    bn[:, 2] = inv[128:256]
    bn[:, 3] = t[128:256]
    return bn


def _get_runner():
    if "runner" in _CACHE:
        return _CACHE["runner"]
    runner = _make_runner(_build_program())
    _CACHE["runner"] = runner
    return runner


def _make_runner(nc):
    from jax.sharding import Mesh, PartitionSpec, NamedSharding
    from jax.experimental.shard_map import shard_map
    from concourse import bass2jax

    bass2jax.install_neuronx_cc_hook()

    partition_name = (
        nc.partition_id_tensor.name if nc.partition_id_tensor is not None else None
    )
    in_names = []
    out_names = []
    out_avals = []
    for alloc in nc.m.functions[0].allocations:
        if not isinstance(alloc, mybir.MemoryLocationSet):
            continue
        name = alloc.memorylocations[0].name
        if alloc.kind == "ExternalInput":
            if name != partition_name:
                in_names.append(name)
        elif alloc.kind == "ExternalOutput":
            out_names.append(name)
            out_avals.append(
                jax.core.ShapedArray(
                    tuple(alloc.tensor_shape), mybir.dt.np(alloc.dtype)
                )
            )
    n_params = len(in_names)
    bind_in_names = tuple(
        in_names + out_names + ([partition_name] if partition_name else [])
    )

    def _body(*args):
        operands = list(args)
        if partition_name is not None:
            operands.append(bass2jax.partition_id_tensor())
        outs = bass2jax._bass_exec_p.bind(
            *operands,
            out_avals=tuple(out_avals),
            in_names=bind_in_names,
            out_names=tuple(out_names),
            lowering_input_output_aliases=(),
            sim_require_finite=True,
            sim_require_nnan=True,
            nc=nc,
        )
        return tuple(outs)

    devices = jax.devices()[:N_CORES]
    mesh = Mesh(np.asarray(devices), ("core",))
    spec = PartitionSpec("core")
    n_out = len(out_names)
    sharded = jax.jit(
        shard_map(
            _body,
            mesh=mesh,
            in_specs=(spec,) * (n_params + n_out),
            out_specs=(spec,) * n_out,
            check_rep=False,
        ),
        keep_unused=True,
    )
    sharding = NamedSharding(mesh, spec)
    zeros = [
        jax.device_put(
            np.zeros((N_CORES * a.shape[0], *a.shape[1:]), a.dtype), sharding
        )
        for a in out_avals
    ]
    return dict(
        nc=nc,
        fn=sharded,
        in_names=in_names,
        out_names=out_names,
        sharding=sharding,
        zeros=zeros,
    )


def _device_inputs(x, weight, gamma, beta, running_mean, running_var):
    """Host-side packing -> concatenated per-core arrays on the 8 devices."""
    r = _get_runner()
    wt = np.asarray(_pack_weights(np.asarray(weight, np.float32)))
    bn = _pack_bn(
        np.asarray(gamma, np.float32),
        np.asarray(beta, np.float32),
        np.asarray(running_mean, np.float32),
        np.asarray(running_var, np.float32),
    )
    x = np.ascontiguousarray(np.asarray(x, np.float32))
    concat = {
        "x": x.reshape(N_CORES * IMG, C, H, W),
        "wt": np.concatenate([wt] * N_CORES, axis=0),
        "bn": np.concatenate([bn] * N_CORES, axis=0),
    }
    args = [
        jax.device_put(concat[name], r["sharding"]) for name in r["in_names"]
    ]
    return r, args


def kernel(x, weight, gamma, beta, running_mean, running_var):
    r, args = _device_inputs(x, weight, gamma, beta, running_mean, running_var)
    outs = r["fn"](*args, *r["zeros"])
    y = np.asarray(outs[0])
    return y.reshape(N_CORES * IMG, C, H, W)


def bench(x, weight, gamma, beta, running_mean, running_var, iters=30):
    """Steady-state per-call wall time (s) with device-resident inputs."""
    import time

    r, args = _device_inputs(x, weight, gamma, beta, running_mean, running_var)
    out = r["fn"](*args, *r["zeros"])
    jax.block_until_ready(out)
    t0 = time.perf_counter()
    for _ in range(iters):
        out = r["fn"](*args, *r["zeros"])
    jax.block_until_ready(out)
    dt = (time.perf_counter() - t0) / iters
    return dt, np.asarray(out[0]).reshape(N_CORES * IMG, C, H, W)


def _time_runner(r, args, iters):
    import time

    out = r["fn"](*args, *r["zeros"])
    jax.block_until_ready(out)
    best = float("inf")
    for _ in range(3):
        t0 = time.perf_counter()
        for _ in range(iters):
            out = r["fn"](*args, *r["zeros"])
        jax.block_until_ready(out)
        best = min(best, (time.perf_counter() - t0) / iters)
    return best, out


def measure_hw_time(
    x, weight, gamma, beta, running_mean, running_var, r_hi=5, iters=40
):
    """Per-launch HW time via repeat-factor slope: T = (t(R) - t(1)) / (R-1).

    Immune to the axon dispatch floor. Returns (hw_seconds, output).
    """
    r1, args = _device_inputs(x, weight, gamma, beta, running_mean, running_var)
    key = f"runner_rep{r_hi}"
    if key not in _CACHE:
        _CACHE[key] = _make_runner(_build_program(repeat=r_hi))
    rH = _CACHE[key]
    t1, out1 = _time_runner(r1, args, iters)
    tH, outH = _time_runner(rH, args, iters)
    hw = (tH - t1) / (r_hi - 1)
    y = np.asarray(out1[0]).reshape(N_CORES * IMG, C, H, W)
    yH = np.asarray(outH[0]).reshape(N_CORES * IMG, C, H, W)
    assert np.array_equal(y, yH), "repeat variant output mismatch"
    return hw, t1, tH, y



# revision 2
# speedup vs baseline: 1.4312x; 1.4312x over previous
"""Trainium2 Bass kernel for nn_BinaryGroupConv — single-x-load + fp8 DoubleRow.

Reference op (per image): BatchNorm2d (inference) -> sign-binarize ->
grouped 3x3 conv (64 groups, 4->4 ch, binarized weights) -> channel
shuffle -> residual add.

Strategy vs the previous (two-load, bf16) kernel:
  - DMA is the wall: the per-core DMA pool sustains ~360 GB/s shared between
    loads and stores, so the old 39 MB of traffic (x loaded twice) cost
    ~108 us. This kernel loads x once: 25.7 MB total = the HW floor.
  - The psum output chunks are regrouped by shuffle-destination (chunk d =
    conv outputs landing in y channels [128d, 128d+128)), so the residual
    add is partition-aligned against the NATURAL x buffer and each psum
    tile stores as one contiguous 128-partition DMA. With this grouping a
    psum chunk contracts over ALL 256 input channels.
  - The K=256 contraction maps exactly onto one fp8 DoubleRow matmul per
    tap: the two k-tiles are the two natural 128-channel input blocks
    (k-tile stride = padded grid size 3376, a multiple of 16 as the HW
    requires). Sign values are exact in fp8e4; PE runs at 2 cols/cycle.
  - Engine split so every elementwise pass runs full-width: Pool (gpsimd)
    does the BN multiply (it cannot touch PSUM, so it gets the SBUF-only
    pass; exact f32 round, separate from the add as in the eager
    reference), ACT does Sign(xs + t) straight to fp8, DVE does the psum+x
    residual add in place into the x buffer, which then doubles as the
    store source. Stores issue from the gpsimd SWDGE queue so they never
    head-of-line-block the x loads on the SP queue.
"""

import numpy as np

import jax
import ml_dtypes

import concourse.bacc as bacc
import concourse.tile as tile
from concourse import mybir
from contextlib import ExitStack

N_CORES = 8
IMG = 4           # images per core
C = 256
H = W = 56
HP = 58           # padded grid row length (1+56+1)
GRIDR = 58        # padded grid rows
BLK = 3376        # k-tile stride: 58*58=3364 padded up to a multiple of 16
APAD = 2 * BLK + 2  # two blocks + 1 guard element each side
HWC = H * W       # 3136
TN = 8 * HP       # 464 matmul free dim per tile (8 output rows)
TN56 = 8 * W      # 448 valid columns per tile
NT = H // 8       # 7 tiles per psum chunk
EPS = 1e-5

_CACHE = {}


def _build_program(repeat=1):
    nc = bacc.Bacc("TRN2")
    f32 = mybir.dt.float32
    fp8 = mybir.dt.float8e4
    x_in = nc.declare_dram_parameter("x", [IMG, C, H, W], f32, isOutput=False)
    wt_in = nc.declare_dram_parameter("wt", [128, 36 * 128], fp8, isOutput=False)
    bn_in = nc.declare_dram_parameter("bn", [128, 4], f32, isOutput=False)
    y_out = nc.declare_dram_parameter("y", [IMG, C, H, W], f32, isOutput=True)

    with tile.TileContext(nc) as tc, ExitStack() as ctx:
        const_pool = ctx.enter_context(tc.tile_pool(name="const", bufs=1))
        ap_pool = ctx.enter_context(tc.tile_pool(name="apb", bufs=1))
        x_pool = ctx.enter_context(tc.tile_pool(name="xin", bufs=4))
        xs_pool = ctx.enter_context(tc.tile_pool(name="xs", bufs=6))
        psum_pool = ctx.enter_context(
            tc.tile_pool(name="ps", bufs=8, space="PSUM")
        )

        # Trigger the ACT table load (Sign set, ~1.3us) immediately so it
        # overlaps the first DMAs instead of the first real activation.
        warm = const_pool.tile([128, 2], f32, tag="actwarm")
        nc.vector.memset(warm[:], 0.0)
        nc.scalar.activation(warm[:], warm[:], mybir.ActivationFunctionType.Sign)

        bn_sb = const_pool.tile([128, 4], f32, tag="bn")
        nc.sync.dma_start(bn_sb[:], bn_in[:])
        wt_sb = const_pool.tile([128, 36 * 128], fp8, tag="wt")
        nc.sync.dma_start(wt_sb[:], wt_in[:])

        # Two round-robin activation buffers; pads/guards zeroed once, the
        # Sign pass rewrites only interiors.
        aps = []
        for i in range(3):
            ap_t = ap_pool.tile([128, APAD], fp8, name=f"ap{i}")
            nc.vector.memset(ap_t[:], 0.0)
            aps.append(ap_t)

        imgs = [i for _rep in range(repeat) for i in range(IMG)]
        st = [None] * len(imgs)
        for k in range(len(imgs)):
            st[k] = _emit_prep(nc, k, imgs[k], x_in, bn_sb, aps, x_pool, xs_pool)
            if k >= 1:
                _emit_compute(nc, st[k - 1], y_out, wt_sb, psum_pool)
        _emit_compute(nc, st[-1], y_out, wt_sb, psum_pool, last=True)
    nc.compile()
    return nc


def _emit_prep(nc, k, img, x_in, bn_sb, aps, x_pool, xs_pool):
    f32 = mybir.dt.float32
    ap_t = aps[k % 3]
    x_t = x_pool.tile([128, 2 * HWC], f32, tag="xt")
    # Natural layout: block b holds channels [128b, 128b+128). 8-row load
    # pieces so BN/sign/matmuls can start while later rows stream in.
    for b, p in [(b, p) for pg in ((0, 2), (2, 4), (4, 6), (6, NT))
                 for b in range(2) for p in range(*pg)]:
        sl = slice(b * HWC + p * TN56, b * HWC + (p + 1) * TN56)
        nc.sync.dma_start(
            x_t[:, sl],
            x_in[img, 128 * b : 128 * (b + 1), 8 * p : 8 * p + 8, :]
            .rearrange("c h w -> c (h w)"),
        )
    # BN+sign, 28-row pieces per block. Two separately-rounded steps
    # (DVE mul, then ACT fused add+Sign) replicate the eager reference's
    # fl(fl(x*inv)+t) bit-exactly; +-1/0 are exact in fp8e4.
    for r0 in (0, 14, 28, 42):
        for b in range(2):
            xs = xs_pool.tile([128, 14 * W], f32, tag="xs")
            src = x_t[:, b * HWC + r0 * W : b * HWC + (r0 + 14) * W]
            nc.gpsimd.tensor_scalar_mul(xs[:], src, bn_sb[:, b : b + 1])
            off = 1 + b * BLK + (r0 + 1) * HP + 1
            interior = ap_t[:, off : off + 14 * HP].rearrange(
                "p (h w) -> p h w", w=HP
            )[:, :, 0:W]
            nc.scalar.activation(
                interior,
                xs[:].rearrange("p (h w) -> p h w", w=W),
                mybir.ActivationFunctionType.Sign,
                bias=bn_sb[:, 2 + b : 3 + b],
                scale=1.0,
            )
    return (img, ap_t, x_t)


def _emit_compute(nc, stage, y_out, wt_sb, psum_pool, last=False):
    img, ap_t, x_t = stage
    f32 = mybir.dt.float32
    # psum chunk d = conv outputs destined for y channels [128d, 128d+128).
    # Each tap is ONE DoubleRow matmul contracting both 128-channel input
    # blocks (k-tiles at stride BLK).
    store_rows = (
        {1: (0, 16), 3: (16, 32), 5: (32, 48), 6: (48, 56)}
        if last
        else {3: (0, 32), 6: (32, 56)}
    )
    for d in range(2):
        for t in range(NT):
            ps = psum_pool.tile([128, TN], f32, tag="ps")
            for tap in range(9):
                dh, dw = tap // 3 - 1, tap % 3 - 1
                s = 1 + HP * (8 * t + 1 + dh) + dw
                rv = ap_t[:, s : s + 1].copy()
                rv.ap = rv.ap[:1] + [[BLK, 2], [1, TN]]
                nc.tensor.matmul(
                    ps[:],
                    wt_sb[:, (d * 9 + tap) * 256 : (d * 9 + tap + 1) * 256]
                    .rearrange("p (two m) -> p two m", two=2),
                    rv,
                    start=(tap == 0),
                    stop=(tap == 8),
                    perf_mode=mybir.MatmulPerfMode.DoubleRow,
                )
            # Residual add on DVE, in place into x_t (the store src).
            xv = x_t[:, d * HWC + t * TN56 : d * HWC + (t + 1) * TN56].rearrange(
                "p (h w) -> p h w", w=W
            )
            pv = ps[:].rearrange("p (h w) -> p h w", w=HP)[:, :, 1 : 1 + W]
            nc.vector.tensor_tensor(xv, pv, xv, op=mybir.AluOpType.add)
            if t in store_rows:
                r0, r1 = store_rows[t]
                nc.gpsimd.dma_start(
                    y_out[img, 128 * d : 128 * (d + 1), r0:r1, :]
                    .rearrange("c h w -> c (h w)"),
                    x_t[:, d * HWC + r0 * W : d * HWC + r1 * W],
                )


def _pack_weights(weight):
    """lhsT tiles [k, d, tap, ktile, m] with shuffle folded into columns.

    psum chunk d, partition m holds conv output oc = 4*(m%64) + (2d + m//64)
    (which channel-shuffles to y channel 128d+m). Its 4 input channels
    4*(m%64)+j live in k-tile b = (m%64)//32 at k = 4*((m%64)%32)+j.
    """
    ws = np.sign(weight.astype(np.float32))  # [256, 4, 3, 3]
    wt = np.zeros((128, 2, 9, 2, 128), np.float32)
    m = np.arange(128)
    c1 = m % 64
    bb = c1 // 32
    kb = 4 * (c1 % 32)
    for d in range(2):
        oc = 4 * c1 + 2 * d + m // 64
        for tap in range(9):
            kh, kw = tap // 3, tap % 3
            for j in range(4):
                wt[kb + j, d, tap, bb, m] = ws[oc, j, kh, kw]
    return wt.reshape(128, 36 * 128).astype(ml_dtypes.float8_e4m3)


def _pack_bn(gamma, beta, running_mean, running_var):
    # Mirror the reference ops (and platform) bit-for-bit.
    import jax.numpy as jnp

    inv = np.asarray(
        jnp.asarray(gamma) * jax.lax.rsqrt(jnp.asarray(running_var) + EPS)
    )
    t = np.asarray(jnp.asarray(beta) - jnp.asarray(running_mean) * jnp.asarray(inv))
    bn = np.zeros((128, 4), np.float32)
    bn[:, 0] = inv[0:128]
    bn[:, 1] = inv[128:256]
    bn[:, 2] = t[0:128]
    bn[:, 3] = t[128:256]
    return bn


def _get_runner():
    if "runner" in _CACHE:
        return _CACHE["runner"]
    runner = _make_runner(_build_program())
    _CACHE["runner"] = runner
    return runner


def _make_runner(nc):
    from jax.sharding import Mesh, PartitionSpec, NamedSharding
    from jax.experimental.shard_map import shard_map
    from concourse import bass2jax

    bass2jax.install_neuronx_cc_hook()

    partition_name = (
        nc.partition_id_tensor.name if nc.partition_id_tensor is not None else None
    )
    in_names = []
    out_names = []
    out_avals = []
    for alloc in nc.m.functions[0].allocations:
        if not isinstance(alloc, mybir.MemoryLocationSet):
            continue
        name = alloc.memorylocations[0].name
        if alloc.kind == "ExternalInput":
            if name != partition_name:
                in_names.append(name)
        elif alloc.kind == "ExternalOutput":
            out_names.append(name)
            out_avals.append(
                jax.core.ShapedArray(
                    tuple(alloc.tensor_shape), mybir.dt.np(alloc.dtype)
                )
            )
    n_params = len(in_names)
    bind_in_names = tuple(
        in_names + out_names + ([partition_name] if partition_name else [])
    )

    def _body(*args):
        operands = list(args)
        if partition_name is not None:
            operands.append(bass2jax.partition_id_tensor())
        outs = bass2jax._bass_exec_p.bind(
            *operands,
            out_avals=tuple(out_avals),
            in_names=bind_in_names,
            out_names=tuple(out_names),
            lowering_input_output_aliases=(),
            sim_require_finite=True,
            sim_require_nnan=True,
            nc=nc,
        )
        return tuple(outs)

    devices = jax.devices()[:N_CORES]
    mesh = Mesh(np.asarray(devices), ("core",))
    spec = PartitionSpec("core")
    n_out = len(out_names)
    sharded = jax.jit(
        shard_map(
            _body,
            mesh=mesh,
            in_specs=(spec,) * (n_params + n_out),
            out_specs=(spec,) * n_out,
            check_rep=False,
        ),
        keep_unused=True,
    )
    sharding = NamedSharding(mesh, spec)
    zeros = [
        jax.device_put(
            np.zeros((N_CORES * a.shape[0], *a.shape[1:]), a.dtype), sharding
        )
        for a in out_avals
    ]
    return dict(
        nc=nc,
        fn=sharded,
        in_names=in_names,
        out_names=out_names,
        sharding=sharding,
        zeros=zeros,
    )


def _device_inputs(x, weight, gamma, beta, running_mean, running_var):
    """Host-side packing -> concatenated per-core arrays on the 8 devices."""
    r = _get_runner()
    wt = np.asarray(_pack_weights(np.asarray(weight, np.float32)))
    bn = _pack_bn(
        np.asarray(gamma, np.float32),
        np.asarray(beta, np.float32),
        np.asarray(running_mean, np.float32),
        np.asarray(running_var, np.float32),
    )
    x = np.ascontiguousarray(np.asarray(x, np.float32))
    concat = {
        "x": x.reshape(N_CORES * IMG, C, H, W),
        "wt": np.concatenate([wt] * N_CORES, axis=0),
        "bn": np.concatenate([bn] * N_CORES, axis=0),
    }
    args = [
        jax.device_put(concat[name], r["sharding"]) for name in r["in_names"]
    ]
    return r, args


def kernel(x, weight, gamma, beta, running_mean, running_var):
    r, args = _device_inputs(x, weight, gamma, beta, running_mean, running_var)
    outs = r["fn"](*args, *r["zeros"])
    y = np.asarray(outs[0])
    return y.reshape(N_CORES * IMG, C, H, W)


def bench(x, weight, gamma, beta, running_mean, running_var, iters=30):
    """Steady-state per-call wall time (s) with device-resident inputs."""
    import time

    r, args = _device_inputs(x, weight, gamma, beta, running_mean, running_var)
    out = r["fn"](*args, *r["zeros"])
    jax.block_until_ready(out)
    t0 = time.perf_counter()
    for _ in range(iters):
        out = r["fn"](*args, *r["zeros"])
    jax.block_until_ready(out)
    dt = (time.perf_counter() - t0) / iters
    return dt, np.asarray(out[0]).reshape(N_CORES * IMG, C, H, W)


def _time_runner(r, args, iters):
    import time

    out = r["fn"](*args, *r["zeros"])
    jax.block_until_ready(out)
    best = float("inf")
    for _ in range(3):
        t0 = time.perf_counter()
        for _ in range(iters):
            out = r["fn"](*args, *r["zeros"])
        jax.block_until_ready(out)
        best = min(best, (time.perf_counter() - t0) / iters)
    return best, out


def measure_hw_time(
    x, weight, gamma, beta, running_mean, running_var, r_hi=5, iters=40
):
    """Per-launch HW time via repeat-factor slope: T = (t(R) - t(1)) / (R-1).

    Immune to the axon dispatch floor. Returns (hw_seconds, output).
    """
    r1, args = _device_inputs(x, weight, gamma, beta, running_mean, running_var)
    key = f"runner_rep{r_hi}"
    if key not in _CACHE:
        _CACHE[key] = _make_runner(_build_program(repeat=r_hi))
    rH = _CACHE[key]
    t1, out1 = _time_runner(r1, args, iters)
    tH, outH = _time_runner(rH, args, iters)
    hw = (tH - t1) / (r_hi - 1)
    y = np.asarray(out1[0]).reshape(N_CORES * IMG, C, H, W)
    yH = np.asarray(outH[0]).reshape(N_CORES * IMG, C, H, W)
    assert np.array_equal(y, yH), "repeat variant output mismatch"
    return hw, t1, tH, y
